# revision 28
# baseline (speedup 1.0000x reference)
"""Trainium2 Bass kernel for nn_NeuralTensorDiagLayer.

Computes out = tanh(concat([e1, e2], -1) @ V + diag + b) where
diag[k] = (sum_b(e1*e2) @ W[k]) / (B*D), broadcast over batch.

Sharding (8 NeuronCores, 2D: 4 batch groups x 2 k_out halves):
  - Core c handles batch rows [1024*(c//2), 1024*(c//2+1)) and k_out
    columns [1024*(c%2), 1024*(c%2+1)).
  - All big streams are bf16 (host casts): X^T resident 8 MiB, V 8 MiB,
    W^T 1 MiB, out 2 MiB -> 19 MiB HBM traffic vs 109 us of PE work
    (bf16 matmul, 1 col/cycle @2.4GHz) => PE-bound design.
  - DMA count is minimized (the HWDGE descriptor path costs ~0.6 us per
    DMA and was the hidden serializer): V arrives as 32 pre-packed
    [128, 1024] SBUF images (4 f-steps each), X as 16 [128, 2048]
    pair-images interleaved into group 0's stream so the TensorEngine
    starts within ~2 us, W^T as a single [128, 4096] image.
  - diag: fused-on-DVE partial sum_b(e1*e2) per core (bf16), AllReduce
    over all 8 cores (8 KiB, 0.5 folded into the scale for the
    double-counted rows), 16 bf16 [1,256] matmuls against W^T in a
    dedicated PSUM bank, AllGather over subgroups [[0,2,4,6],[1,3,5,7]]
    assembles each k_out half (slice assignment permuted host-side, see
    make_in_maps). The 16 PE matmuls sit between groups 1 and 2 in the
    in-order PE stream (not after group 0) so the PE never waits on the
    collective; drains of groups 0/1 are emitted after the chain so
    def-before-use holds for the diag bias.
  - Main loop: k-tile groups (2,1,2,1,1,1) -> (4,2,4,2,2,2) PSUM banks
    from a 7-bank pool; current + draining group never exceed 7 banks so
    the PE never stalls on PSUM, and the final 1-ktile groups shorten the
    serial drain tail. DVE drains PSUM to a bf16 stage (unconditional,
    fast) so the PE is decoupled from the diag collective chain; ScalarE
    applies tanh with the diag+b column as per-partition bias; out is
    written bf16 and upcast on the host.
  - Measured (tick-forced wall-clock slope, R=1 vs R=33): 83.1 us/pass +
    17.3 us sim lead-in => ~100 us vs 439 us baseline.

Output is produced transposed ([k_out, batch] per core); the host
transposes/concats the 4x2 block grid back to (B, K).
"""

import os
import sys

for _p in ("/opt/trn_rl_repo", "/root/.axon_site/_ro/trn_rl_repo"):
    if os.path.isdir(_p) and _p not in sys.path:
        sys.path.append(_p)

import numpy as np

N_CORES = 8
B, D, K_OUT = 4096, 2048, 2048
FEAT = 2 * D
BG, KH = 4, 2                 # batch groups x kout halves
BPC = B // BG                 # 1024 batch rows per core
KHC = K_OUT // KH             # 1024 kout cols per core
KPC = K_OUT // N_CORES        # 256 diag rows per core
FT = FEAT // 128              # 32 feature tiles
DT = D // 128                 # 16 e1-space feature tiles
KTL = KHC // 128              # 8 local kout tiles
KGROUPS = (2, 1, 2, 1, 1, 1)  # kout tiles per group (2x = live PSUM banks)
DIAG_SCALE = 0.5 / (B * D)    # 0.5: the 8-core allreduce double-counts rows

_CACHE = {}


def _build_nc():
    import concourse.bacc as bacc
    import concourse.tile as tile
    import concourse.mybir as mybir

    repeat = int(os.environ.get("KERNEL_REPEAT", "1"))
    no_cc = bool(int(os.environ.get("KERNEL_NO_CC", "0")))
    skip_diag = bool(int(os.environ.get("KERNEL_SKIP_DIAG", "0")))
    with_tick = bool(int(os.environ.get("KERNEL_TICK", "0")))
    dt = mybir.dt
    nc = bacc.Bacc("TRN2", target_bir_lowering=False, debug=False,
                   num_devices=N_CORES)

    # x pair-images: row-block jp is the SBUF image [128, 2*BPC] holding
    # f-tiles (2jp, 2jp+1); V megatile-images: row-block m of vp{g} is the
    # SBUF image [128, 1024] holding that group's f-steps 4m..4m+3;
    # W^T image: [128, DT*KPC].
    xp = nc.dram_tensor("xp", [DT * 128, 2 * BPC], dt.bfloat16,
                        kind="ExternalInput").ap()
    vps = [nc.dram_tensor(f"vp{g}", [FT * gw * 16, 1024], dt.bfloat16,
                          kind="ExternalInput").ap()
           for g, gw in enumerate(KGROUPS)]
    wtp = nc.dram_tensor("wtp", [128, DT * KPC], dt.bfloat16,
                         kind="ExternalInput").ap()
    bvec = nc.dram_tensor("bvec", [1, KPC], dt.float32, kind="ExternalInput").ap()
    out = nc.dram_tensor("out", [KHC, BPC], dt.bfloat16, kind="ExternalOutput").ap()
    tick = (nc.dram_tensor("tick", [1, KTL], dt.bfloat16,
                           kind="ExternalOutput").ap() if with_tick else None)

    core_ids = list(range(N_CORES))
    ag_groups = [[0, 2, 4, 6], [1, 3, 5, 7]]

    with tile.TileContext(nc) as tc:
        with tc.tile_pool(name="xpool", bufs=1) as xpool, \
             tc.tile_pool(name="vpool", bufs=4) as vpool, \
             tc.tile_pool(name="spool", bufs=1) as spool, \
             tc.tile_pool(name="scratch", bufs=2) as scratch, \
             tc.tile_pool(name="stage", bufs=1) as stage_pool, \
             tc.tile_pool(name="opool", bufs=2) as opool, \
             tc.tile_pool(name="psum", bufs=7, space="PSUM") as pp, \
             tc.tile_pool(name="psd", bufs=1, space="PSUM") as ppd, \
             tc.tile_pool(name="dram", bufs=1, space="DRAM") as dram:

            # pair 0 is split into two single-tile halves so the first
            # matmul's moving operand lands in ~1 us (deps are per-tile)
            x0a = xpool.tile([128, BPC], dt.bfloat16, name="x0a")
            x0b = xpool.tile([128, BPC], dt.bfloat16, name="x0b")
            x_pairs = [None] + [
                xpool.tile([128, 2 * BPC], dt.bfloat16, name=f"xq{jp}")
                for jp in range(1, DT)]

            def x_sl(ft, lo, hi):
                if ft == 0:
                    return x0a[:, lo:hi]
                if ft == 1:
                    return x0b[:, lo:hi]
                t = x_pairs[ft // 2]
                off = (ft % 2) * BPC
                return t[:, off + lo:off + hi]

            s_sb = spool.tile([128, DT], dt.float32)
            diag_cols = spool.tile([128, KTL], dt.float32, name="diag_cols")

            def emit_x_pair(jp):
                """DMA x pair-image jp; then any DVE diag partials whose two
                f-tiles are now resident (pairs 8.. complete e2-side)."""
                if jp == 0:
                    nc.sync.dma_start(x0a[:], xp[0:128, 0:BPC])
                    nc.sync.dma_start(x0b[:], xp[0:128, BPC:2 * BPC])
                else:
                    nc.sync.dma_start(x_pairs[jp][:],
                                      xp[jp * 128:(jp + 1) * 128, :])
                if not skip_diag and jp >= DT // 2:
                    for j in (2 * (jp - DT // 2), 2 * (jp - DT // 2) + 1):
                        prod = scratch.tile([128, BPC], dt.bfloat16,
                                            tag="prod", name=f"prod{j}")
                        nc.vector.tensor_mul(
                            prod[:], x_sl(j, 0, BPC), x_sl(DT + j, 0, BPC))
                        nc.vector.tensor_reduce(s_sb[:, j:j + 1], prod[:],
                                                mybir.AxisListType.X,
                                                mybir.AluOpType.add)

            def emit_diag_chain():
                # AllReduce s -> [1,KPC] diag slice -> AllGather -> diag_cols
                if skip_diag:
                    return
                b_sb = spool.tile([1, KPC], dt.float32, name="b_sb")
                nc.sync.dma_start(b_sb[:], bvec[:])
                s_in = dram.tile([128, DT], dt.float32)
                s_out = dram.tile([128, DT], dt.float32,
                                  addr_space="Local" if no_cc else "Shared")
                nc.sync.dma_start(s_in[:], s_sb[:])
                if no_cc:
                    nc.sync.dma_start(s_out[:], s_in[:])
                else:
                    nc.gpsimd.collective_compute(
                        "AllReduce", mybir.AluOpType.add,
                        replica_groups=[core_ids],
                        ins=[s_in.opt()], outs=[s_out.opt()])
                s_r = spool.tile([128, DT], dt.float32, name="s_r")
                nc.sync.dma_start(s_r[:], s_out[:])
                s_b = spool.tile([128, DT], dt.bfloat16, name="s_bf")
                nc.vector.tensor_copy(s_b[:], s_r[:])

                w_img = spool.tile([128, DT * KPC], dt.bfloat16, name="w_img")
                nc.sync.dma_start(w_img[:], wtp[:])
                diag_sb = spool.tile([1, KPC], dt.float32, name="diag_sb")
                ps_d = ppd.tile([1, KPC], dt.float32)
                for j in range(DT):
                    nc.tensor.matmul(ps_d[:], s_b[:, j:j + 1],
                                     w_img[:, j * KPC:(j + 1) * KPC],
                                     start=(j == 0), stop=(j == DT - 1))
                nc.vector.tensor_scalar_mul(diag_sb[:], ps_d[:], DIAG_SCALE)
                nc.vector.tensor_add(diag_sb[:], diag_sb[:], b_sb[:])

                d_in = dram.tile([1, KPC], dt.float32, name="d_in")
                d_out = dram.tile([KTL, 128], dt.float32, name="d_out")
                nc.sync.dma_start(d_in[:], diag_sb[:])
                if no_cc:
                    for i in range(4):
                        nc.sync.dma_start(
                            d_out[2 * i:2 * i + 2, :],
                            d_in[:].rearrange("a (x p) -> (a x) p", p=128))
                else:
                    nc.gpsimd.collective_compute(
                        "AllGather", mybir.AluOpType.bypass,
                        replica_groups=ag_groups,
                        ins=[d_in.opt()], outs=[d_out.opt()])
                # load as [128, KTL]: partition p, col k <- diag_half[k*128+p]
                nc.sync.dma_start(diag_cols[:],
                                  d_out[:].rearrange("k p -> p k"))

            # ---- main matmul: out^T = V_half^T @ X^T, bf16 on TensorE ----
            stage = stage_pool.tile([128, KTL * BPC], dt.bfloat16, name="stage")

            def emit_copies(k0, gw, pss):
                for q in range(gw):
                    kt = k0 + q
                    for b2 in range(2):
                        nc.vector.tensor_copy(
                            stage[:, kt * BPC + b2 * 512:
                                  kt * BPC + (b2 + 1) * 512],
                            pss[q][b2][:])

            def emit_act_out(_rep, k0, gw):
                for q in range(gw):
                    kt = k0 + q
                    ot = opool.tile([128, BPC], dt.bfloat16, tag="ot",
                                    name=f"ot{_rep}_{kt}")
                    bias_kw = {} if skip_diag else {
                        "bias": diag_cols[:, kt:kt + 1]}
                    nc.scalar.activation(
                        ot[:], stage[:, kt * BPC:(kt + 1) * BPC],
                        mybir.ActivationFunctionType.Tanh, **bias_kw)
                    nc.sync.dma_start(out[kt * 128:(kt + 1) * 128, :], ot[:])
                    if with_tick and _rep == repeat - 1:
                        nc.sync.dma_start(tick[0:1, kt:kt + 1], ot[0:1, 0:1])

            def emit_drain(_rep, k0, gw, pss, direct=False):
                # direct=True: ScalarE reads PSUM straight (skips the DVE
                # stage hop). Only safe when nothing downstream needs the
                # banks soon - used for the final rep's last groups, where
                # the drain chain is the serial program tail.
                if not direct:
                    emit_copies(k0, gw, pss)
                    emit_act_out(_rep, k0, gw)
                    return
                for q in range(gw):
                    kt = k0 + q
                    # per-b2 ot tiles + out DMAs so the b2=0 writeback
                    # overlaps the b2=1 activation in the program tail
                    bias_kw = {} if skip_diag else {
                        "bias": diag_cols[:, kt:kt + 1]}
                    for b2 in range(2):
                        oth = opool.tile([128, 512], dt.bfloat16, tag="oth",
                                         name=f"ot{_rep}_{kt}_{b2}")
                        nc.scalar.activation(
                            oth[:], pss[q][b2][:],
                            mybir.ActivationFunctionType.Tanh, **bias_kw)
                        nc.sync.dma_start(
                            out[kt * 128:(kt + 1) * 128,
                                b2 * 512:(b2 + 1) * 512], oth[:])
                        if with_tick and _rep == repeat - 1 and b2 == 1:
                            nc.sync.dma_start(tick[0:1, kt:kt + 1],
                                              oth[0:1, 0:1])

            def emit_group(_rep, kg, gw, k0):
                """One steady-state k-tile group: stream V megatiles,
                accumulate, return psum tiles (drain emitted by caller)."""
                pss = [[pp.tile([128, 512], dt.float32, tag="ps",
                                name=f"ps{_rep}_{kg}_{q}_{b2}")
                        for b2 in range(2)] for q in range(gw)]
                vp = vps[kg]
                per = 8 // gw             # f-tiles per megatile
                for m in range(FT * gw // 8):
                    vt = vpool.tile([128, 1024], dt.bfloat16, tag="vt",
                                    name=f"vt{_rep}_{kg}_{m}")
                    nc.sync.dma_start(vt[:], vp[m * 128:(m + 1) * 128, :])
                    for i in range(per):
                        ft = per * m + i
                        for q in range(gw):
                            stat = vt[:, (i * gw + q) * 128:
                                      (i * gw + q + 1) * 128]
                            for b2 in range(2):
                                nc.tensor.matmul(
                                    pss[q][b2][:], stat,
                                    x_sl(ft, b2 * 512, (b2 + 1) * 512),
                                    start=(ft == 0), stop=(ft == FT - 1))
                return pss

            def emit_fused_first_pass():
                """Rep 0, ktiles 0-2 fused (groups 0+1): 6 matmuls per
                f-tile outpace the interleaved x+V DMA stream, so the PE is
                never x-starved during the 8 MiB x load. 6 PSUM banks."""
                pss01 = [[pp.tile([128, 512], dt.float32, tag="ps",
                                  name=f"psf_{q}_{b2}")
                          for b2 in range(2)] for q in range(2)]
                pss2 = [[pp.tile([128, 512], dt.float32, tag="ps",
                                 name=f"psf_2_{b2}")
                         for b2 in range(2)]]
                vp0, vp1 = vps[0], vps[1]
                vt1 = None
                for m in range(8):        # vp0 megatiles, 4 f-tiles each
                    if m == 0:
                        # split first megatile + first x pair so the very
                        # first matmul's operands land in ~1 us
                        vt_a = spool.tile([128, 256], dt.bfloat16,
                                          name="vt0a")
                        nc.sync.dma_start(vt_a[:], vp0[0:128, 0:256])
                        nc.sync.dma_start(x0a[:], xp[0:128, 0:BPC])
                        vt_b = spool.tile([128, 768], dt.bfloat16,
                                          name="vt0b")
                        nc.sync.dma_start(vt_b[:], vp0[0:128, 256:1024])
                        nc.sync.dma_start(x0b[:], xp[0:128, BPC:2 * BPC])
                        emit_x_pair(1)

                        def stats01(i, q):
                            if i == 0:
                                return vt_a[:, q * 128:(q + 1) * 128]
                            return vt_b[:, (i - 1) * 256 + q * 128:
                                        (i - 1) * 256 + (q + 1) * 128]
                    else:
                        vt = vpool.tile([128, 1024], dt.bfloat16, tag="vt",
                                        name=f"vtf_{m}")
                        nc.sync.dma_start(vt[:], vp0[m * 128:(m + 1) * 128, :])
                        emit_x_pair(2 * m)
                        emit_x_pair(2 * m + 1)

                        def stats01(i, q, vt=vt):
                            return vt[:, (i * 2 + q) * 128:
                                      (i * 2 + q + 1) * 128]
                    if m % 2 == 0:
                        vt1 = vpool.tile([128, 1024], dt.bfloat16, tag="vt",
                                         name=f"vtf1_{m // 2}")
                        nc.sync.dma_start(
                            vt1[:], vp1[(m // 2) * 128:(m // 2 + 1) * 128, :])
                    for i in range(4):
                        ft = 4 * m + i
                        i1 = ft % 8       # position within vt1
                        for q in range(2):
                            stat = stats01(i, q)
                            for b2 in range(2):
                                nc.tensor.matmul(
                                    pss01[q][b2][:], stat,
                                    x_sl(ft, b2 * 512, (b2 + 1) * 512),
                                    start=(ft == 0), stop=(ft == FT - 1))
                        stat1 = vt1[:, i1 * 128:(i1 + 1) * 128]
                        for b2 in range(2):
                            nc.tensor.matmul(
                                pss2[0][b2][:], stat1,
                                x_sl(ft, b2 * 512, (b2 + 1) * 512),
                                start=(ft == 0), stop=(ft == FT - 1))
                return pss01, pss2

            for _rep in range(repeat):
              if _rep == 0:
                # fused ktiles 0-2; engine-queue ordering: fused DVE copies
                # precede the chain's DVE ops so PSUM frees fast; the
                # chain's 16 PE matmuls land after group 2's matmuls so the
                # PE never waits on the AllReduce; ACTs come after the
                # chain so def-before-use holds for diag_cols.
                pss01, pss2 = emit_fused_first_pass()
                emit_copies(0, 2, pss01)
                emit_copies(2, 1, pss2)
                kg, gw, k0 = 2, KGROUPS[2], 3
                pss_g2 = emit_group(_rep, kg, gw, k0)
                emit_diag_chain()
                emit_act_out(_rep, 0, 2)
                emit_act_out(_rep, 2, 1)
                emit_drain(_rep, k0, gw, pss_g2,
                           direct=(repeat == 1 and kg >= len(KGROUPS) - 2))
                k0 = 3 + gw
                start_kg = 3
              else:
                k0 = 0
                start_kg = 0
              for kg in range(start_kg, len(KGROUPS)):
                gw = KGROUPS[kg]
                pss = emit_group(_rep, kg, gw, k0)
                emit_drain(_rep, k0, gw, pss,
                           direct=(_rep == repeat - 1 and
                                   kg >= len(KGROUPS) - 2))
                k0 += gw

    nc.compile()
    return nc


def _get_nc():
    if "nc" not in _CACHE:
        _CACHE["nc"] = _build_nc()
    return _CACHE["nc"]


def _bf16(x):
    import ml_dtypes
    return np.ascontiguousarray(x).astype(ml_dtypes.bfloat16)


def _pack_x(e1s, e2s):
    """e1s/e2s: [BPC, D] f32 slices -> [DT*128, 2*BPC] bf16 pair-images."""
    xt = np.concatenate([_bf16(e1s.T), _bf16(e2s.T)], axis=0)  # [FEAT, BPC]
    tiles = xt.reshape(FT, 128, BPC)
    pairs = np.concatenate([tiles[0::2], tiles[1::2]], axis=2)  # [16,128,2BPC]
    return np.ascontiguousarray(pairs.reshape(DT * 128, 2 * BPC))


def _pack_v(v_half_bf):
    """v_half_bf: [FEAT, KHC] bf16 -> per-group megatile images.

    gw==2 group at cols c0:c0+256: megatile m image [128, 1024] holds
    f-tiles 4m..4m+3 side by side (each [128, 256]).
    gw==1 group at cols c0:c0+128: megatile m image holds f-tiles
    8m..8m+7 (each [128, 128])."""
    packed = []
    k0 = 0
    for gw in KGROUPS:
        cols = v_half_bf[:, k0 * 128:(k0 + gw) * 128]     # [FEAT, gw*128]
        blk = cols.reshape(FT, 128, gw * 128)
        per = 8 // gw                                      # f-tiles per mega
        nmega = FT // per
        img = np.concatenate([blk[i::per] for i in range(per)], axis=2)
        packed.append(np.ascontiguousarray(img.reshape(nmega * 128, 1024)))
        k0 += gw
    return packed


def make_in_maps(e1, e2, W, V, b):
    in_maps = []
    Wb = _bf16(W)
    Vb = _bf16(V)
    for c in range(N_CORES):
        g, h = c // 2, c % 2
        sc = h * 4 + g            # permuted diag-slice index (see module doc)
        rows = slice(g * BPC, (g + 1) * BPC)
        krows = slice(sc * KPC, (sc + 1) * KPC)
        # W^T image: [128, DT*KPC], block j = W^T[j*128:(j+1)*128, :]
        wt = np.ascontiguousarray(Wb[krows].T)             # [D, KPC]
        w_img = np.ascontiguousarray(
            wt.reshape(DT, 128, KPC).transpose(1, 0, 2).reshape(128, DT * KPC))
        vpacks = _pack_v(np.ascontiguousarray(Vb[:, h * KHC:(h + 1) * KHC]))
        im = {
            "xp": _pack_x(e1[rows], e2[rows]),
            "wtp": w_img,
            "bvec": np.ascontiguousarray(b[krows].reshape(1, KPC),
                                         dtype=np.float32),
        }
        for gi, vp in enumerate(vpacks):
            im[f"vp{gi}"] = vp
        in_maps.append(im)
    return in_maps


def kernel(e1, e2, W, V, b):
    from concourse.bass_utils import run_bass_kernel_spmd

    e1 = np.asarray(e1, dtype=np.float32)
    e2 = np.asarray(e2, dtype=np.float32)
    W = np.asarray(W, dtype=np.float32)
    V = np.asarray(V, dtype=np.float32)
    b = np.asarray(b, dtype=np.float32)

    nc = _get_nc()
    res = run_bass_kernel_spmd(nc, make_in_maps(e1, e2, W, V, b),
                               list(range(N_CORES)))
    out = np.empty((B, K_OUT), dtype=np.float32)
    for c in range(N_CORES):
        g, h = c // 2, c % 2
        out[g * BPC:(g + 1) * BPC, h * KHC:(h + 1) * KHC] = \
            res.results[c]["out"].astype(np.float32).T
    return out


# revision 30
# speedup vs baseline: 1.3566x; 1.3566x over previous
"""Trainium2 Bass kernel for nn_NeuralTensorDiagLayer.

Computes out = tanh(concat([e1, e2], -1) @ V + diag + b) where
diag[k] = (sum_b(e1*e2) @ W[k]) / (B*D), broadcast over batch.

Sharding (8 NeuronCores, 2D: 4 batch groups x 2 k_out halves):
  - Core c handles batch rows [1024*(c//2), 1024*(c//2+1)) and k_out
    columns [1024*(c%2), 1024*(c%2+1)).
  - All big streams are bf16 (host casts): X^T resident 8 MiB, V 8 MiB,
    W^T 1 MiB, out 2 MiB -> 19 MiB HBM traffic vs 109 us of PE work
    (bf16 matmul, 1 col/cycle @2.4GHz) => PE-bound design.
  - DMA count is minimized (the HWDGE descriptor path costs ~0.6 us per
    DMA and was the hidden serializer): V arrives as 32 pre-packed
    [128, 1024] SBUF images (4 f-steps each), X as 16 [128, 2048]
    pair-images interleaved into group 0's stream so the TensorEngine
    starts within ~2 us, W^T as a single [128, 4096] image.
  - diag: fused-on-DVE partial sum_b(e1*e2) per core (bf16), AllReduce
    over all 8 cores (8 KiB, 0.5 folded into the scale for the
    double-counted rows), 16 bf16 [1,256] matmuls against W^T in a
    dedicated PSUM bank, AllGather over subgroups [[0,2,4,6],[1,3,5,7]]
    assembles each k_out half (slice assignment permuted host-side, see
    make_in_maps). The 16 PE matmuls sit between groups 1 and 2 in the
    in-order PE stream (not after group 0) so the PE never waits on the
    collective; drains of groups 0/1 are emitted after the chain so
    def-before-use holds for the diag bias.
  - Main loop: k-tile groups (2,1,2,1,1,1) -> (4,2,4,2,2,2) PSUM banks
    from a 7-bank pool; current + draining group never exceed 7 banks so
    the PE never stalls on PSUM, and the final 1-ktile groups shorten the
    serial drain tail. DVE drains PSUM to a bf16 stage (unconditional,
    fast) so the PE is decoupled from the diag collective chain; ScalarE
    applies tanh with the diag+b column as per-partition bias; out is
    written bf16 and upcast on the host.
  - Measured (tick-forced wall-clock slope, R=1 vs R=33): 83.1 us/pass +
    17.3 us sim lead-in => ~100 us vs 439 us baseline.

Output is produced transposed ([k_out, batch] per core); the host
transposes/concats the 4x2 block grid back to (B, K).
"""

import os
import sys

for _p in ("/opt/trn_rl_repo", "/root/.axon_site/_ro/trn_rl_repo"):
    if os.path.isdir(_p) and _p not in sys.path:
        sys.path.append(_p)

import numpy as np

N_CORES = 8
B, D, K_OUT = 4096, 2048, 2048
FEAT = 2 * D
BG, KH = 4, 2                 # batch groups x kout halves
BPC = B // BG                 # 1024 batch rows per core
KHC = K_OUT // KH             # 1024 kout cols per core
KPC = K_OUT // N_CORES        # 256 diag rows per core
FT = FEAT // 128              # 32 feature tiles
DT = D // 128                 # 16 e1-space feature tiles
KTL = KHC // 128              # 8 local kout tiles
KGROUPS = (2, 1, 2, 1, 1, 1)  # kout tiles per group (2x = live PSUM banks)
DIAG_SCALE = 0.5 / (B * D)    # 0.5: the 8-core allreduce double-counts rows

_CACHE = {}


def _build_nc():
    import concourse.bacc as bacc
    import concourse.tile as tile
    import concourse.mybir as mybir

    repeat = int(os.environ.get("KERNEL_REPEAT", "1"))
    no_cc = bool(int(os.environ.get("KERNEL_NO_CC", "0")))
    skip_diag = bool(int(os.environ.get("KERNEL_SKIP_DIAG", "0")))
    with_tick = bool(int(os.environ.get("KERNEL_TICK", "0")))
    dt = mybir.dt
    nc = bacc.Bacc("TRN2", target_bir_lowering=False, debug=False,
                   num_devices=N_CORES)

    # x pair-images: row-block jp is the SBUF image [128, 2*BPC] holding
    # f-tiles (2jp, 2jp+1); V megatile-images: row-block m of vp{g} is the
    # SBUF image [128, 1024] holding that group's f-steps 4m..4m+3;
    # W^T image: [128, DT*KPC].
    xp = nc.dram_tensor("xp", [DT * 128, 2 * BPC], dt.bfloat16,
                        kind="ExternalInput").ap()
    vps = [nc.dram_tensor(f"vp{g}", [FT * gw * 16, 1024], dt.bfloat16,
                          kind="ExternalInput").ap()
           for g, gw in enumerate(KGROUPS)]
    wtp = nc.dram_tensor("wtp", [128, DT * KPC], dt.bfloat16,
                         kind="ExternalInput").ap()
    bvec = nc.dram_tensor("bvec", [1, KPC], dt.float32, kind="ExternalInput").ap()
    out = nc.dram_tensor("out", [KHC, BPC], dt.bfloat16, kind="ExternalOutput").ap()
    tick = (nc.dram_tensor("tick", [1, KTL], dt.bfloat16,
                           kind="ExternalOutput").ap() if with_tick else None)

    core_ids = list(range(N_CORES))
    ag_groups = [[0, 2, 4, 6], [1, 3, 5, 7]]

    with tile.TileContext(nc) as tc:
        with tc.tile_pool(name="xpool", bufs=1) as xpool, \
             tc.tile_pool(name="vpool", bufs=4) as vpool, \
             tc.tile_pool(name="spool", bufs=1) as spool, \
             tc.tile_pool(name="scratch", bufs=2) as scratch, \
             tc.tile_pool(name="stage", bufs=1) as stage_pool, \
             tc.tile_pool(name="opool", bufs=2) as opool, \
             tc.tile_pool(name="psum", bufs=7, space="PSUM") as pp, \
             tc.tile_pool(name="psd", bufs=1, space="PSUM") as ppd, \
             tc.tile_pool(name="dram", bufs=1, space="DRAM") as dram:

            # pair 0 is split into two single-tile halves so the first
            # matmul's moving operand lands in ~1 us (deps are per-tile)
            x0a = xpool.tile([128, BPC], dt.bfloat16, name="x0a")
            x0b = xpool.tile([128, BPC], dt.bfloat16, name="x0b")
            x_pairs = [None] + [
                xpool.tile([128, 2 * BPC], dt.bfloat16, name=f"xq{jp}")
                for jp in range(1, DT)]

            def x_sl(ft, lo, hi):
                if ft == 0:
                    return x0a[:, lo:hi]
                if ft == 1:
                    return x0b[:, lo:hi]
                t = x_pairs[ft // 2]
                off = (ft % 2) * BPC
                return t[:, off + lo:off + hi]

            s_sb = spool.tile([128, DT], dt.float32)
            diag_cols = spool.tile([128, KTL], dt.float32, name="diag_cols")

            def emit_x_pair(jp):
                """DMA x pair-image jp; then any DVE diag partials whose two
                f-tiles are now resident (pairs 8.. complete e2-side)."""
                if jp == 0:
                    nc.sync.dma_start(x0a[:], xp[0:128, 0:BPC])
                    nc.sync.dma_start(x0b[:], xp[0:128, BPC:2 * BPC])
                else:
                    nc.sync.dma_start(x_pairs[jp][:],
                                      xp[jp * 128:(jp + 1) * 128, :])
                if not skip_diag and jp >= DT // 2:
                    for j in (2 * (jp - DT // 2), 2 * (jp - DT // 2) + 1):
                        prod = scratch.tile([128, BPC], dt.bfloat16,
                                            tag="prod", name=f"prod{j}")
                        nc.vector.tensor_mul(
                            prod[:], x_sl(j, 0, BPC), x_sl(DT + j, 0, BPC))
                        nc.vector.tensor_reduce(s_sb[:, j:j + 1], prod[:],
                                                mybir.AxisListType.X,
                                                mybir.AluOpType.add)

            def emit_diag_chain():
                # AllReduce s -> [1,KPC] diag slice -> AllGather -> diag_cols
                if skip_diag:
                    return
                b_sb = spool.tile([1, KPC], dt.float32, name="b_sb")
                nc.sync.dma_start(b_sb[:], bvec[:])
                s_in = dram.tile([128, DT], dt.float32)
                s_out = dram.tile([128, DT], dt.float32,
                                  addr_space="Local" if no_cc else "Shared")
                nc.sync.dma_start(s_in[:], s_sb[:])
                if no_cc:
                    nc.sync.dma_start(s_out[:], s_in[:])
                else:
                    nc.gpsimd.collective_compute(
                        "AllReduce", mybir.AluOpType.add,
                        replica_groups=[core_ids],
                        ins=[s_in.opt()], outs=[s_out.opt()])
                s_r = spool.tile([128, DT], dt.float32, name="s_r")
                nc.sync.dma_start(s_r[:], s_out[:])
                s_b = spool.tile([128, DT], dt.bfloat16, name="s_bf")
                nc.vector.tensor_copy(s_b[:], s_r[:])

                w_img = spool.tile([128, DT * KPC], dt.bfloat16, name="w_img")
                nc.sync.dma_start(w_img[:], wtp[:])
                diag_sb = spool.tile([1, KPC], dt.float32, name="diag_sb")
                ps_d = ppd.tile([1, KPC], dt.float32)
                for j in range(DT):
                    nc.tensor.matmul(ps_d[:], s_b[:, j:j + 1],
                                     w_img[:, j * KPC:(j + 1) * KPC],
                                     start=(j == 0), stop=(j == DT - 1))
                nc.vector.tensor_scalar_mul(diag_sb[:], ps_d[:], DIAG_SCALE)
                nc.vector.tensor_add(diag_sb[:], diag_sb[:], b_sb[:])

                d_in = dram.tile([1, KPC], dt.float32, name="d_in")
                d_out = dram.tile([KTL, 128], dt.float32, name="d_out")
                nc.sync.dma_start(d_in[:], diag_sb[:])
                if no_cc:
                    for i in range(4):
                        nc.sync.dma_start(
                            d_out[2 * i:2 * i + 2, :],
                            d_in[:].rearrange("a (x p) -> (a x) p", p=128))
                else:
                    nc.gpsimd.collective_compute(
                        "AllGather", mybir.AluOpType.bypass,
                        replica_groups=ag_groups,
                        ins=[d_in.opt()], outs=[d_out.opt()])
                # load as [128, KTL]: partition p, col k <- diag_half[k*128+p]
                nc.sync.dma_start(diag_cols[:],
                                  d_out[:].rearrange("k p -> p k"))

            # ---- main matmul: out^T = V_half^T @ X^T, bf16 on TensorE ----
            stage = stage_pool.tile([128, KTL * BPC], dt.bfloat16, name="stage")

            def emit_copies(k0, gw, pss):
                for q in range(gw):
                    kt = k0 + q
                    for b2 in range(2):
                        nc.vector.tensor_copy(
                            stage[:, kt * BPC + b2 * 512:
                                  kt * BPC + (b2 + 1) * 512],
                            pss[q][b2][:])

            def emit_act_out(_rep, k0, gw):
                for q in range(gw):
                    kt = k0 + q
                    ot = opool.tile([128, BPC], dt.bfloat16, tag="ot",
                                    name=f"ot{_rep}_{kt}")
                    bias_kw = {} if skip_diag else {
                        "bias": diag_cols[:, kt:kt + 1]}
                    nc.scalar.activation(
                        ot[:], stage[:, kt * BPC:(kt + 1) * BPC],
                        mybir.ActivationFunctionType.Tanh, **bias_kw)
                    nc.sync.dma_start(out[kt * 128:(kt + 1) * 128, :], ot[:])
                    if with_tick and _rep == repeat - 1:
                        nc.sync.dma_start(tick[0:1, kt:kt + 1], ot[0:1, 0:1])

            def emit_drain(_rep, k0, gw, pss, direct=False):
                # direct=True: ScalarE reads PSUM straight (skips the DVE
                # stage hop). Only safe when nothing downstream needs the
                # banks soon - used for the final rep's last groups, where
                # the drain chain is the serial program tail.
                if not direct:
                    emit_copies(k0, gw, pss)
                    emit_act_out(_rep, k0, gw)
                    return
                for q in range(gw):
                    kt = k0 + q
                    # per-b2 ot tiles + out DMAs so the b2=0 writeback
                    # overlaps the b2=1 activation in the program tail
                    bias_kw = {} if skip_diag else {
                        "bias": diag_cols[:, kt:kt + 1]}
                    for b2 in range(2):
                        oth = opool.tile([128, 512], dt.bfloat16, tag="oth",
                                         name=f"ot{_rep}_{kt}_{b2}")
                        nc.scalar.activation(
                            oth[:], pss[q][b2][:],
                            mybir.ActivationFunctionType.Tanh, **bias_kw)
                        nc.sync.dma_start(
                            out[kt * 128:(kt + 1) * 128,
                                b2 * 512:(b2 + 1) * 512], oth[:])
                        if with_tick and _rep == repeat - 1 and b2 == 1:
                            nc.sync.dma_start(tick[0:1, kt:kt + 1],
                                              oth[0:1, 0:1])

            def emit_group(_rep, kg, gw, k0):
                """One steady-state k-tile group: stream V megatiles,
                accumulate, return psum tiles (drain emitted by caller)."""
                pss = [[pp.tile([128, 512], dt.float32, tag="ps",
                                name=f"ps{_rep}_{kg}_{q}_{b2}")
                        for b2 in range(2)] for q in range(gw)]
                vp = vps[kg]
                per = 8 // gw             # f-tiles per megatile
                for m in range(FT * gw // 8):
                    vt = vpool.tile([128, 1024], dt.bfloat16, tag="vt",
                                    name=f"vt{_rep}_{kg}_{m}")
                    nc.sync.dma_start(vt[:], vp[m * 128:(m + 1) * 128, :])
                    for i in range(per):
                        ft = per * m + i
                        for q in range(gw):
                            stat = vt[:, (i * gw + q) * 128:
                                      (i * gw + q + 1) * 128]
                            for b2 in range(2):
                                nc.tensor.matmul(
                                    pss[q][b2][:], stat,
                                    x_sl(ft, b2 * 512, (b2 + 1) * 512),
                                    start=(ft == 0), stop=(ft == FT - 1))
                return pss

            def emit_fused_first_pass():
                """Rep 0, ktiles 0-2 fused (groups 0+1): 6 matmuls per
                f-tile outpace the interleaved x+V DMA stream, so the PE is
                never x-starved during the 8 MiB x load. 6 PSUM banks."""
                pss01 = [[pp.tile([128, 512], dt.float32, tag="ps",
                                  name=f"psf_{q}_{b2}")
                          for b2 in range(2)] for q in range(2)]
                pss2 = [[pp.tile([128, 512], dt.float32, tag="ps",
                                 name=f"psf_2_{b2}")
                         for b2 in range(2)]]
                vp0, vp1 = vps[0], vps[1]
                vt1 = None
                for m in range(8):        # vp0 megatiles, 4 f-tiles each
                    if m == 0:
                        # split first megatile + first x pair so the very
                        # first matmul's operands land in ~1 us
                        vt_a = spool.tile([128, 256], dt.bfloat16,
                                          name="vt0a")
                        nc.sync.dma_start(vt_a[:], vp0[0:128, 0:256])
                        nc.sync.dma_start(x0a[:], xp[0:128, 0:BPC])
                        vt1 = vpool.tile([128, 1024], dt.bfloat16, tag="vt",
                                         name="vtf1_0")
                        nc.sync.dma_start(vt1[:], vp1[0:128, :])
                        vt_b = spool.tile([128, 768], dt.bfloat16,
                                          name="vt0b")
                        nc.sync.dma_start(vt_b[:], vp0[0:128, 256:1024])
                        nc.sync.dma_start(x0b[:], xp[0:128, BPC:2 * BPC])
                        emit_x_pair(1)

                        def stats01(i, q):
                            if i == 0:
                                return vt_a[:, q * 128:(q + 1) * 128]
                            return vt_b[:, (i - 1) * 256 + q * 128:
                                        (i - 1) * 256 + (q + 1) * 128]
                    else:
                        vt = vpool.tile([128, 1024], dt.bfloat16, tag="vt",
                                        name=f"vtf_{m}")
                        nc.sync.dma_start(vt[:], vp0[m * 128:(m + 1) * 128, :])
                        emit_x_pair(2 * m)
                        emit_x_pair(2 * m + 1)

                        def stats01(i, q, vt=vt):
                            return vt[:, (i * 2 + q) * 128:
                                      (i * 2 + q + 1) * 128]
                    if m % 2 == 0 and m > 0:
                        vt1 = vpool.tile([128, 1024], dt.bfloat16, tag="vt",
                                         name=f"vtf1_{m // 2}")
                        nc.sync.dma_start(
                            vt1[:], vp1[(m // 2) * 128:(m // 2 + 1) * 128, :])
                    for i in range(4):
                        ft = 4 * m + i
                        i1 = ft % 8       # position within vt1
                        for q in range(2):
                            stat = stats01(i, q)
                            for b2 in range(2):
                                nc.tensor.matmul(
                                    pss01[q][b2][:], stat,
                                    x_sl(ft, b2 * 512, (b2 + 1) * 512),
                                    start=(ft == 0), stop=(ft == FT - 1))
                        stat1 = vt1[:, i1 * 128:(i1 + 1) * 128]
                        for b2 in range(2):
                            nc.tensor.matmul(
                                pss2[0][b2][:], stat1,
                                x_sl(ft, b2 * 512, (b2 + 1) * 512),
                                start=(ft == 0), stop=(ft == FT - 1))
                return pss01, pss2

            for _rep in range(repeat):
              if _rep == 0:
                # fused ktiles 0-2; engine-queue ordering: fused DVE copies
                # precede the chain's DVE ops so PSUM frees fast; the
                # chain's 16 PE matmuls land after group 2's matmuls so the
                # PE never waits on the AllReduce; ACTs come after the
                # chain so def-before-use holds for diag_cols.
                pss01, pss2 = emit_fused_first_pass()
                emit_copies(0, 2, pss01)
                emit_copies(2, 1, pss2)
                kg, gw, k0 = 2, KGROUPS[2], 3
                pss_g2 = emit_group(_rep, kg, gw, k0)
                emit_diag_chain()
                emit_act_out(_rep, 0, 2)
                emit_act_out(_rep, 2, 1)
                emit_drain(_rep, k0, gw, pss_g2,
                           direct=(repeat == 1 and kg >= len(KGROUPS) - 2))
                k0 = 3 + gw
                start_kg = 3
              else:
                k0 = 0
                start_kg = 0
              for kg in range(start_kg, len(KGROUPS)):
                gw = KGROUPS[kg]
                pss = emit_group(_rep, kg, gw, k0)
                emit_drain(_rep, k0, gw, pss,
                           direct=(_rep == repeat - 1 and
                                   kg >= len(KGROUPS) - 2))
                k0 += gw

    nc.compile()
    return nc


def _get_nc():
    if "nc" not in _CACHE:
        _CACHE["nc"] = _build_nc()
    return _CACHE["nc"]


def _bf16(x):
    import ml_dtypes
    return np.ascontiguousarray(x).astype(ml_dtypes.bfloat16)


def _pack_x(e1s, e2s):
    """e1s/e2s: [BPC, D] f32 slices -> [DT*128, 2*BPC] bf16 pair-images."""
    xt = np.concatenate([_bf16(e1s.T), _bf16(e2s.T)], axis=0)  # [FEAT, BPC]
    tiles = xt.reshape(FT, 128, BPC)
    pairs = np.concatenate([tiles[0::2], tiles[1::2]], axis=2)  # [16,128,2BPC]
    return np.ascontiguousarray(pairs.reshape(DT * 128, 2 * BPC))


def _pack_v(v_half_bf):
    """v_half_bf: [FEAT, KHC] bf16 -> per-group megatile images.

    gw==2 group at cols c0:c0+256: megatile m image [128, 1024] holds
    f-tiles 4m..4m+3 side by side (each [128, 256]).
    gw==1 group at cols c0:c0+128: megatile m image holds f-tiles
    8m..8m+7 (each [128, 128])."""
    packed = []
    k0 = 0
    for gw in KGROUPS:
        cols = v_half_bf[:, k0 * 128:(k0 + gw) * 128]     # [FEAT, gw*128]
        blk = cols.reshape(FT, 128, gw * 128)
        per = 8 // gw                                      # f-tiles per mega
        nmega = FT // per
        img = np.concatenate([blk[i::per] for i in range(per)], axis=2)
        packed.append(np.ascontiguousarray(img.reshape(nmega * 128, 1024)))
        k0 += gw
    return packed


def make_in_maps(e1, e2, W, V, b):
    in_maps = []
    Wb = _bf16(W)
    Vb = _bf16(V)
    for c in range(N_CORES):
        g, h = c // 2, c % 2
        sc = h * 4 + g            # permuted diag-slice index (see module doc)
        rows = slice(g * BPC, (g + 1) * BPC)
        krows = slice(sc * KPC, (sc + 1) * KPC)
        # W^T image: [128, DT*KPC], block j = W^T[j*128:(j+1)*128, :]
        wt = np.ascontiguousarray(Wb[krows].T)             # [D, KPC]
        w_img = np.ascontiguousarray(
            wt.reshape(DT, 128, KPC).transpose(1, 0, 2).reshape(128, DT * KPC))
        vpacks = _pack_v(np.ascontiguousarray(Vb[:, h * KHC:(h + 1) * KHC]))
        im = {
            "xp": _pack_x(e1[rows], e2[rows]),
            "wtp": w_img,
            "bvec": np.ascontiguousarray(b[krows].reshape(1, KPC),
                                         dtype=np.float32),
        }
        for gi, vp in enumerate(vpacks):
            im[f"vp{gi}"] = vp
        in_maps.append(im)
    return in_maps


def kernel(e1, e2, W, V, b):
    from concourse.bass_utils import run_bass_kernel_spmd

    e1 = np.asarray(e1, dtype=np.float32)
    e2 = np.asarray(e2, dtype=np.float32)
    W = np.asarray(W, dtype=np.float32)
    V = np.asarray(V, dtype=np.float32)
    b = np.asarray(b, dtype=np.float32)

    nc = _get_nc()
    res = run_bass_kernel_spmd(nc, make_in_maps(e1, e2, W, V, b),
                               list(range(N_CORES)))
    out = np.empty((B, K_OUT), dtype=np.float32)
    for c in range(N_CORES):
        g, h = c // 2, c % 2
        out[g * BPC:(g + 1) * BPC, h * KHC:(h + 1) * KHC] = \
            res.results[c]["out"].astype(np.float32).T
    return out
